# revision 5
# baseline (speedup 1.0000x reference)
"""Int4LinearDequant Trainium2 kernel — fp8 DoubleRow version.

Computes y = x @ dequant(qweight, scale).T + bias for
  x:       [4, 2048, 4096] fp32
  qweight: [11008, 2048]   int32 (one uint8 byte per element, two int4 nibbles)
  scale:   [11008]         fp32
  bias:    [11008]         fp32
  y:       [4, 2048, 11008] fp32

Strategy (column-parallel across 8 cores; fp8 DoubleRow matmuls):
  - Each core owns 1376 of the 11008 out_features.
  - All quantization/packing happens on the host: int4 nibbles are unpacked
    to exact fp8(e4m3) integer weights; x is scaled by 4 and split into
    x_hi = e4m3(4x) plus x_lo = e4m3(4x - x_hi).
  - The PE runs fp8 DoubleRow matmuls (K=256 per call, 2 fp8 weights/cell,
    measured 2.0x bf16 throughput with LDWEIGHTS fully hidden).  Each
    DoubleRow "pair" holds two k-tile slots:
      * corrected pair  : slots = (x_hi[t], x_lo[t]) against duplicated
        weights (W[t], W[t]) -> contributes W[t]*(x_hi+x_lo), i.e. x at
        double-fp8 precision (error ~0.07%).  The weight duplication is
        free: the moving AP's slot dimension uses stride 0.
      * uncorrected pair: slots = (x_hi[t1], x_hi[t2]) against
        (W[t1], W[t2]) -> plain e4m3 precision for those k-tiles.
    With H of the 32 k-tiles corrected, rel err ~= 2.56% * sqrt((32-H)/32)
    and the matmul cost is (32+H)/2 pair-calls per output tile.
  - A short warmup stream of dummy matmuls keeps the PE busy while the
    first DMAs land so the HAM clock gate opens at ~3.4us instead of
    tens of us into the kernel.
  - Epilogue applies scale/4 (the 1/4 undoes the x prescale) and bias on
    DVE, then DMA out in natural [tokens, out] layout.
"""

import os
import sys

import numpy as np

sys.path.insert(0, "/opt/trn_rl_repo")

import ml_dtypes

E4 = ml_dtypes.float8_e4m3

N_CORES = 8
IN_F = 4096
OUT_F = 11008
PACKED = IN_F // 2  # 2048
B, S = 4, 2048
TOK = B * S  # 8192
OUT_SH = OUT_F // N_CORES  # 1376
P = 128
M_TILES = TOK // P  # 64
K_TILES = IN_F // P  # 32
CB_TILES = PACKED // P  # 16
N_SPLITS = [(0, 512), (512, 512), (1024, 352)]

H = 14  # corrected k-tiles (must be even; pairs = 16 + H//2)
N_PAIRS = CB_TILES + H // 2
N_WARMUP = 30

_cache: dict = {}


def _pair_slot_tiles():
    """k-tile index feeding each (pair, slot) and whether it is the lo part."""
    tiles = np.empty((N_PAIRS, 2), dtype=np.int64)
    is_lo = np.zeros((N_PAIRS, 2), dtype=bool)
    for cb in range(H // 2):
        tiles[2 * cb] = (cb, cb)
        is_lo[2 * cb] = (False, True)
        tiles[2 * cb + 1] = (CB_TILES + cb, CB_TILES + cb)
        is_lo[2 * cb + 1] = (False, True)
    for cb in range(H // 2, CB_TILES):
        j = H + (cb - H // 2)
        tiles[j] = (cb, CB_TILES + cb)
    return tiles, is_lo


def _build_program():
    if "nc" in _cache:
        return _cache["nc"]

    from concourse import bacc, mybir
    import concourse.bass as bass
    import concourse.tile as tile

    f32 = mybir.dt.float32
    fp8 = mybir.dt.float8e4
    PM = mybir.MatmulPerfMode

    nc = bacc.Bacc("TRN2", target_bir_lowering=False, debug=False)

    xh = nc.dram_tensor("xh", [M_TILES, P, N_PAIRS, 2, P], fp8, kind="ExternalInput")
    qw = nc.dram_tensor("qw", [P, K_TILES * OUT_SH], fp8, kind="ExternalInput")
    scale = nc.dram_tensor("scale", [OUT_SH], f32, kind="ExternalInput")
    bias = nc.dram_tensor("bias", [OUT_SH], f32, kind="ExternalInput")
    out = nc.dram_tensor("out", [M_TILES, P, OUT_SH], f32, kind="ExternalOutput")

    tiles, _ = _pair_slot_tiles()

    def bcast_rows(ap_1d, nparts):
        # DMA access pattern that reads the same DRAM row for every partition
        return bass.AP(
            tensor=ap_1d.tensor,
            offset=ap_1d.offset,
            ap=[[0, nparts]] + list(ap_1d.ap),
        )

    def pair_rhs(qw_tile, j, off, nsz):
        # moving AP [128, 2, nsz]: slot s reads W[tiles[j][s]] columns off:off+nsz
        a, b = int(tiles[j][0]), int(tiles[j][1])
        base = qw_tile[:, a, off : off + nsz]
        return bass.AP(
            tensor=base.tensor,
            offset=base.offset,
            ap=[list(base.ap[0]), [(b - a) * OUT_SH, 2], list(base.ap[1])],
        )

    with tile.TileContext(nc) as tc:
        with (
            tc.tile_pool(name="qmat", bufs=1) as qmat,
            tc.tile_pool(name="xp", bufs=6) as xp,
            tc.tile_pool(name="outp", bufs=4) as outp,
            tc.tile_pool(name="psA", bufs=3, space="PSUM") as psA,
            tc.tile_pool(name="psB", bufs=3, space="PSUM") as psB,
            tc.tile_pool(name="psC", bufs=2, space="PSUM") as psC,
        ):
            ps_pools = {0: psA, 512: psB, 1024: psC}

            # HAM warmup: dummy matmuls with no dependency at all keep the
            # PE busy while input DMAs stream, so the clock gate opens
            # early.  The operand is a raw (uninitialized) SBUF tensor:
            # its values are never observed -- the warmup PSUM bank is
            # never read, and every real accumulation group begins with
            # start=True which overwrites PSUM regardless of content.
            warm_t = nc.alloc_sbuf_tensor("warmt", [P, 2, 512], fp8)
            warm = warm_t.ap()
            WGRP = 6
            for g in range(N_WARMUP // WGRP):
                wps = psA.tile([P, 512], f32, name="ps0")
                for i in range(WGRP):
                    nc.tensor.matmul(
                        wps[:],
                        lhsT=warm[:, :, 0:128],
                        rhs=warm,
                        start=(i == 0),
                        stop=(i == WGRP - 1),
                        perf_mode=PM.DoubleRow,
                    )

            # first x tile ahead of the weight stream so PE can start ASAP
            xt0 = xp.tile([P, N_PAIRS, 2, P], fp8, name="xt")
            nc.sync.dma_start(out=xt0[:], in_=xh[0])

            # resident fp8 weights [128, 32 k-tiles, OUT_SH].  The sync DMA
            # queue is serial, so interleave the early x-tile prefetches
            # between the weight sub-chunks: weights for the first splits
            # land first, and x1..x3 are not starved behind the whole
            # weight stream.
            QW = qmat.tile([P, K_TILES, OUT_SH], fp8)
            scale_rep = qmat.tile([P, OUT_SH], f32)
            bias_rep = qmat.tile([P, OUT_SH], f32)
            xts = {0: xt0}
            for m in (1, 2, 3):
                xts[m] = xp.tile([P, N_PAIRS, 2, P], fp8, name="xt")

            def qw_sub(ci, k0):
                off, nsz = N_SPLITS[ci]
                qstart = sum(K_TILES * n for _, n in N_SPLITS[:ci])
                nc.sync.dma_start(
                    out=QW[:, k0 : k0 + 8, off : off + nsz],
                    in_=qw.ap()[:, qstart + k0 * nsz : qstart + (k0 + 8) * nsz],
                )

            qw_sub(0, 0); qw_sub(0, 16)
            nc.sync.dma_start(out=xts[1][:], in_=xh[1])
            qw_sub(0, 8); qw_sub(0, 24)
            nc.sync.dma_start(out=xts[2][:], in_=xh[2])
            qw_sub(1, 0); qw_sub(1, 16)
            nc.sync.dma_start(out=xts[3][:], in_=xh[3])
            qw_sub(1, 8); qw_sub(1, 24)
            qw_sub(2, 0); qw_sub(2, 16)
            nc.sync.dma_start(out=scale_rep[:], in_=bcast_rows(scale.ap(), P))
            qw_sub(2, 8); qw_sub(2, 24)
            nc.sync.dma_start(out=bias_rep[:], in_=bcast_rows(bias.ap(), P))

            def do_split(xt, m, off, nsz):
                ps = ps_pools[off].tile([P, 512], f32, name=f"ps{off}")
                for j in range(N_PAIRS):
                    nc.tensor.matmul(
                        ps[:, :nsz],
                        lhsT=xt[:, j, :, :],
                        rhs=pair_rhs(QW, j, off, nsz),
                        start=(j == 0),
                        stop=(j == N_PAIRS - 1),
                        perf_mode=PM.DoubleRow,
                    )
                ot = outp.tile([P, 512], f32, name="ot")
                nc.vector.tensor_tensor(
                    out=ot[:, :nsz], in0=ps[:, :nsz],
                    in1=scale_rep[:, off : off + nsz], op=mybir.AluOpType.mult,
                )
                nc.vector.tensor_tensor(
                    out=ot[:, :nsz], in0=ot[:, :nsz],
                    in1=bias_rep[:, off : off + nsz], op=mybir.AluOpType.add,
                )
                nc.sync.dma_start(
                    out=out[m][:, off : off + nsz], in_=ot[:, :nsz]
                )

            # first three m-tiles: 512-splits first (QW tail chunk still in
            # flight), then their 352-splits
            for m, (off, nsz) in [
                (0, N_SPLITS[0]), (0, N_SPLITS[1]),
                (1, N_SPLITS[0]), (1, N_SPLITS[1]),
                (2, N_SPLITS[0]), (2, N_SPLITS[1]),
                (0, N_SPLITS[2]), (1, N_SPLITS[2]), (2, N_SPLITS[2]),
            ]:
                do_split(xts[m], m, off, nsz)

            for m in range(3, M_TILES):
                if m in xts:
                    xt = xts[m]
                else:
                    xt = xp.tile([P, N_PAIRS, 2, P], fp8, name="xt")
                    nc.sync.dma_start(out=xt[:], in_=xh[m])
                for off, nsz in N_SPLITS:
                    do_split(xt, m, off, nsz)

    nc.compile()
    _cache["nc"] = nc
    return nc


def _prep_x(x: np.ndarray) -> np.ndarray:
    """[4,2048,4096] fp32 -> [64, 128, N_PAIRS, 2, 128] e4m3 pair tiles.

    k-tile t covers input columns 2*((t%16)*128+p) + t//16 (even inputs for
    t<16 from low nibbles, odd for t>=16 from high), transposed so inputs sit
    on SBUF partitions.  Slots hold x_hi = e4m3(4x) and, for corrected
    tiles, x_lo = e4m3(4x - x_hi).
    """
    xs = (x.reshape(TOK, IN_F) * np.float32(4.0)).astype(np.float32)
    hi = xs.astype(E4)
    lo = (xs - hi.astype(np.float32)).astype(E4)

    def tile4(a):
        # [tok, in] -> [m, p, kt, j] with kt = e*16 + cb
        a4 = a.reshape(M_TILES, P, CB_TILES, P, 2)  # [m, j, cb, p, e]
        return np.ascontiguousarray(a4.transpose(0, 3, 4, 2, 1)).reshape(
            M_TILES, P, K_TILES, P
        )

    hi4 = tile4(hi)  # [m, p, kt, j]
    lo4 = tile4(lo)

    tiles, is_lo = _pair_slot_tiles()
    # [m, p, N_PAIRS, 2, j]
    sel_hi = hi4[:, :, tiles, :]
    sel_lo = lo4[:, :, tiles, :]
    out = np.where(is_lo[None, None, :, :, None], sel_lo, sel_hi)
    return np.ascontiguousarray(out)


def _prep_w(qweight: np.ndarray, core: int) -> np.ndarray:
    """Core shard of weights as fp8 [128, K_TILES, OUT_SH]."""
    qw = qweight[core * OUT_SH : (core + 1) * OUT_SH].astype(np.uint8)  # [o, 2048]
    low = ((qw & 15).astype(np.int8) - 8).astype(E4)  # [o, packed]
    high = ((qw >> 4).astype(np.int8) - 8).astype(E4)
    # W[kt][p, o]: kt<16 low nibble of packed col kt*128+p; kt>=16 high
    lowT = np.ascontiguousarray(low.T).reshape(CB_TILES, P, OUT_SH)
    highT = np.ascontiguousarray(high.T).reshape(CB_TILES, P, OUT_SH)
    W = np.concatenate([lowT, highT], axis=0)  # [32, p, o]
    Wp = np.ascontiguousarray(W.transpose(1, 0, 2))  # [p, 32, o]
    chunks = [
        Wp[:, :, off : off + nsz].reshape(P, -1)
        for off, nsz in N_SPLITS
    ]
    return np.ascontiguousarray(np.concatenate(chunks, axis=1))  # [p, 32*o]


def kernel(x, qweight, scale, bias):
    from concourse.bass_utils import run_bass_kernel_spmd
    from concourse.bass_interp import get_hw_module

    nc = _build_program()

    xh = _prep_x(np.asarray(x))
    qweight = np.asarray(qweight)
    scale = np.asarray(scale, dtype=np.float32) * np.float32(0.25)
    bias = np.asarray(bias, dtype=np.float32)

    in_maps = []
    for c in range(N_CORES):
        in_maps.append(
            {
                "xh": xh,
                "qw": _prep_w(qweight, c),
                "scale": scale[c * OUT_SH : (c + 1) * OUT_SH],
                "bias": bias[c * OUT_SH : (c + 1) * OUT_SH],
            }
        )

    old_m = nc.m
    nc.m = get_hw_module(nc.m)
    try:
        res = run_bass_kernel_spmd(
            nc,
            in_maps,
            core_ids=list(range(N_CORES)),
            trace=bool(int(os.environ.get("K_TRACE", "0"))),
            tmpdir=os.environ.get("K_TRACE_DIR") or None,
        )
    finally:
        nc.m = old_m
    _cache["last_results"] = res

    out = np.empty((TOK, OUT_F), dtype=np.float32)
    for c in range(N_CORES):
        out[:, c * OUT_SH : (c + 1) * OUT_SH] = (
            res.results[c]["out"].reshape(TOK, OUT_SH)
        )
    return out.reshape(B, S, OUT_F)
